# revision 32
# baseline (speedup 1.0000x reference)
"""Bass/Tile TRN2 kernel: pairwise-MLP multi-head attention (B=2,T=256,C=128,H=4,HS=32).

Sharding: 8 cores = (batch b in {0,1}) x (query residue k in {0..3}); core
(b, k) owns the 64 queries i == k (mod 4), so every core sees the same mix of
causal extents and the SPMD program is identical across cores.

Causal extents are rounded up to multiples of 32: local query q (global
i = 4q + k) computes E_q = 32*(q//8 + 1) key columns.  All (q, j) pairs are
packed into one flat stream (9216 columns per head-side), so every DoubleRow
pre-matmul block uses the same kt-jump delta:

  pd0cat | pd1cat | x1kcat | x1qcat    (big_sb, fp8; x1k/x1q built on-chip)

Per-core dataflow:
  pre[c,(q,j)] = fp8 DoubleRow matmuls: (W1p_lo | W1k) @ (pd0 | x1k) +
                 (W1p_hi | W1q) @ (pd1 | x1q), 256 rows per PE instruction.
  g = gelu(pre + b1[h])                        (ScalarE, psum -> sbuf bf16)
  score_t[j,q] = g_chunk.T @ w2[h]             (PE, g stationary, bf16)
  P_t = exp(scale*score_t + b2*scale) * mask   (ScalarE + DVE, per head)
  out[q,:] = P_t.T @ [v | 1]; out /= Z         (PE; Z rides as v's 33rd column)

Startup: gelu act-table + PE HAM prewarmed during the input DMA; weights are
packed head-first so head 0's stationaries arrive in the first small chunk.
"""

import sys
from contextlib import ExitStack

import numpy as np

for _p in ("/opt/trn_rl_repo", "/root/.axon_site/_ro/trn_rl_repo"):
    if _p not in sys.path:
        sys.path.append(_p)

import ml_dtypes

import concourse.bass as bass
import concourse.mybir as mybir
import concourse.tile as tile
from concourse.bass_utils import run_bass_kernel_spmd

B, T, C = 2, 256, 128
H, HS = 4, 32
IBLK = 64            # queries per core
NCORES = 8
SCALE = float(C) ** -0.5

F32 = mybir.dt.float32
BF16 = mybir.dt.bfloat16
F8 = mybir.dt.float8e4
DR = mybir.MatmulPerfMode.DoubleRow

GELU = mybir.ActivationFunctionType.Gelu
EXP = mybir.ActivationFunctionType.Exp

# ---- stream layout ----------------------------------------------------
# class c (0..7): local queries 8c..8c+7, extent E = 32*(c+1).
# tiles of 1024 stream columns; no query straddles a tile.
TILE_QS = [
    [56, 57, 58, 59],
    [60, 61, 62, 63],
    [24, 25, 26, 27, 28, 29, 30, 31],
    [48, 49, 50, 51, 0, 1, 2, 3],
    [52, 53, 54, 55, 4, 5, 6, 7],
    [40, 41, 42, 43, 8, 9, 10, 11],
    [44, 45, 46, 47, 12, 13, 14, 15],
    [32, 33, 34, 35, 16, 17, 18, 19],
    [36, 37, 38, 39, 20, 21, 22, 23],
]
NT = len(TILE_QS)


def _ext(q):
    return 32 * (q // 8 + 1)


STREAM_Q = [q for tq in TILE_QS for q in tq]
NS = sum(_ext(q) for q in STREAM_Q)          # 9216 stream cols per side
assert NS == 9216
# per-tile metadata: (q, E, colstart within tile)
TILE_META = []
for tq in TILE_QS:
    pos, meta = 0, []
    for q in tq:
        meta.append((q, _ext(q), pos))
        pos += _ext(q)
    assert pos == 1024
    TILE_META.append(meta)

PD0 = 0
PD1 = NS
XK = 2 * NS
XQ = 3 * NS
NBIG = 4 * NS
DAB = XK - PD0           # 18432: kt-jump delta for both matmuls

# wq layout: wab head-first (c, h, [a/b, kt], m) | w2 (8 fp8 = 4 bf16) |
# b1 (16 fp8 = 4 f32)
OW2 = 2048
OB1 = OW2 + 2 * H
NWQ = OB1 + 4 * H

_build_cache = {}


def _legalize_single_wait(bir_json):
    """Split multi-wait instructions into single-wait NoOps + instruction.

    This walrus build's codegen (setupSyncWait) accepts at most one sem wait
    per ISA struct, but Tile's sem-assignment attaches wait *lists*.  Waits
    are ANDed and executed in order by the issuing sequencer, so hoisting all
    but one onto same-engine NoOps immediately before is semantically
    identical.
    """
    import json as _json

    m = _json.loads(bir_json)
    for fn in m.get("functions", []):
        for blk in fn.get("blocks", []):
            new = []
            for ins in blk.get("instructions", []):
                si = ins.get("sync_info")
                waits = (si or {}).get("on_wait") or []
                if len(waits) > 1:
                    for k, w in enumerate(waits[:-1]):
                        nop = {
                            "debug": ins.get("debug", 0),
                            "engine": ins["engine"],
                            "ins": [],
                            "name": f"{ins['name']}-ws{k}",
                            "opcode": "NoOp",
                            "outs": [],
                            "sync_info": {"on_wait": [w], "on_update": []},
                        }
                        new.append(nop)
                    si = dict(si)
                    si["on_wait"] = [waits[-1]]
                    ins = dict(ins)
                    ins["sync_info"] = si
                new.append(ins)
            blk["instructions"] = new
    return _json.dumps(m).encode()


def _install_wait_legalizer():
    from concourse import bass2jax as _b2j
    from concourse import bass_utils as _bu

    if getattr(_b2j, "_single_wait_patched", False):
        return
    _orig = _bu.compile_bir_kernel

    def _patched(bir_json, tmpdir, neff_name="file.neff"):
        return _orig(_legalize_single_wait(bir_json), tmpdir, neff_name)

    _b2j.compile_bir_kernel = _patched
    _b2j._single_wait_patched = True


def _ktjump(tile_ap, off, delta, ncols):
    """rhs AP [128][kt: stride delta, 2][1, ncols] rooted at column `off`."""
    sl = tile_ap[:, off : off + 1]
    return bass.AP(
        tensor=sl.tensor,
        offset=sl.offset,
        ap=[list(sl.ap[0]), [delta, 2], [1, ncols]],
    )


def _flat(t, lo, n):
    """[128, n] view of columns [lo, lo+n) of a [128, 8, 128] sbuf tile."""
    sl = t[:, lo // 128, lo % 128 : lo % 128 + 1]
    return bass.AP(tensor=sl.tensor, offset=sl.offset,
                   ap=[list(sl.ap[0]), [1, n]])


def _build(b2_scaled):
    nc = bass.Bass()

    big = nc.dram_tensor("big", (128, NBIG), F8, kind="ExternalInput")
    wabq = nc.dram_tensor("wabq", (128, NWQ), F8, kind="ExternalInput")
    cpack = nc.dram_tensor("cpack", (128, 516), BF16, kind="ExternalInput")
    out = nc.dram_tensor("out", (IBLK, H * HS), F32, kind="ExternalOutput")

    with tile.TileContext(nc) as tc, ExitStack() as ctx:
        const = ctx.enter_context(tc.tile_pool(name="const", bufs=1))
        gpool = ctx.enter_context(tc.tile_pool(name="gpool", bufs=4))
        psg = ctx.enter_context(tc.tile_pool(name="psg", bufs=2, space="PSUM"))
        pssc = ctx.enter_context(tc.tile_pool(name="pssc", bufs=1, space="PSUM"))
        psepi = ctx.enter_context(tc.tile_pool(name="psepi", bufs=1, space="PSUM"))
        pswarm = ctx.enter_context(tc.tile_pool(name="pswarm", bufs=1, space="PSUM"))

        wq_sb = const.tile([128, NWQ], F8)
        cp_sb = const.tile([128, 516], BF16)
        big_sb = const.tile([128, NBIG], F8)

        # ---------- prewarm seeds (idle engines at t=0) ----------
        warm = const.tile([128, 16], BF16)
        wscr = const.tile([128, 16], BF16)
        nc.vector.memset(warm, 1.0)
        # gelu ACT_TABLE_LOAD fires before this, overlapping the input DMA
        nc.scalar.activation(out=wscr, in_=warm, func=GELU, scale=1.0)

        # ---------- input DMA: 3 queues, chunks ordered by first use ------
        # scalar: w2/b1 mini-chunk, head-0 weights, rest, cpack
        nc.scalar.dma_start(out=wq_sb[:, OW2:NWQ], in_=wabq[:, OW2:NWQ])
        nc.scalar.dma_start(out=wq_sb[:, 0:512], in_=wabq[:, 0:512])
        nc.scalar.dma_start(out=wq_sb[:, 512:2048], in_=wabq[:, 512:2048])
        nc.scalar.dma_start(out=cp_sb, in_=cpack[:])

        # sync (HW queue): pd0 + x1k; gpsimd (SW queue): pd1 + x1q, tile-major
        chunks = ((0, 256), (256, 512), (512, 768), (768, 1024),
                  (1024, 1280), (1280, 1536), (1536, 2048), (2048, 3072),
                  (3072, 4096), (4096, 5120), (5120, 7168), (7168, 9216))
        for lo, hi in chunks:
            nc.sync.dma_start(out=big_sb[:, PD0 + lo : PD0 + hi],
                              in_=big[:, PD0 + lo : PD0 + hi])
            nc.gpsimd.dma_start(out=big_sb[:, PD1 + lo : PD1 + hi],
                                in_=big[:, PD1 + lo : PD1 + hi])
            nc.sync.dma_start(out=big_sb[:, XK + lo : XK + hi],
                              in_=big[:, XK + lo : XK + hi])
            nc.gpsimd.dma_start(out=big_sb[:, XQ + lo : XQ + hi],
                                in_=big[:, XQ + lo : XQ + hi])

        # ---------- PE HAM warm-up ----------
        wps = pswarm.tile([16, 512], F32)
        wsl16 = warm[:, 0:1]
        warm_rhs = bass.AP(tensor=wsl16.tensor, offset=wsl16.offset,
                           ap=[list(wsl16.ap[0]), [0, 512]])
        for _ in range(10):
            nc.tensor.matmul(wps, lhsT=warm, rhs=warm_rhs, start=True, stop=True)

        msl = cp_sb[:, 388:389]

        def w2_ap(h):
            o = OW2 + 2 * h
            return wq_sb[:, o : o + 2].bitcast(BF16)

        def b1_ap(h):
            o = OB1 + 4 * h
            return wq_sb[:, o : o + 4].bitcast(F32)

        def wsl(ab, h):
            """lhsT view [c, kt, 128] of head h's a/b weight pack."""
            o = h * 512 + ab * 256
            sl = wq_sb[:, o : o + 1]
            return bass.AP(tensor=sl.tensor, offset=sl.offset,
                           ap=[list(sl.ap[0]), [128, 2], [1, 128]])

        pt_sb = const.tile([128, 2, H, IBLK], BF16)
        v_sb = const.tile([128, 2, H, HS + 1], BF16)
        recip = const.tile([IBLK, H, 1], F32)
        final_sb = const.tile([IBLK, H * HS], F32)

        # score accumulator [j%128, jb, h, i] - 1 PSUM bank, memset so the
        # never-written j-tiles of short queries exp() to a finite value.
        score_ps = pssc.tile([128, 2, H, IBLK], F32)
        nc.vector.memset(score_ps, 0.0)
        nc.vector.memset(v_sb[:, :, :, HS : HS + 1], 1.0)

        # ---------- v = x @ Wv (+ ones column for Z) ----------
        for jc in range(2):
            v_ps = psepi.tile([128, H, HS], F32, tag="vps", name=f"v{jc}")
            nc.tensor.matmul(v_ps, lhsT=cp_sb[:, 132 + jc * 128 : 260 + jc * 128],
                             rhs=cp_sb[:, 4:132], start=True, stop=True)
            nc.vector.tensor_copy(v_sb[:, jc, :, 0:HS], v_ps)

        # ---------- main loop ----------
        pending = []

        def emit_scores(g_t, t, h):
            # chunks padded to M=128 where possible (garbage partitions land
            # beyond the causal extent and are killed by the mask) so every
            # LDWEIGHTS is a full 128-col FWL load.
            for q, E, cs in TILE_META[t]:
                m0 = min(128, 1024 - cs)
                nc.tensor.matmul(score_ps[0:m0, 0, h, q : q + 1],
                                 lhsT=_flat(g_t, cs, m0), rhs=w2_ap(h),
                                 start=True, stop=True)
                if E > 128:
                    m1 = min(128, 1024 - cs - 128)
                    nc.tensor.matmul(score_ps[0:m1, 1, h, q : q + 1],
                                     lhsT=_flat(g_t, cs + 128, m1),
                                     rhs=w2_ap(h), start=True, stop=True)

        warm_rhs32 = bass.AP(tensor=wsl16.tensor, offset=wsl16.offset,
                             ap=[list(wsl16.ap[0]), [0, 32]])
        for t in range(NT):
            base = 1024 * t
            # keep-alive during the DMA ramp: tiny dep-free matmul so a
            # data stall cannot leave the PE idle for a full HAM MID window
            if t < 5:
                nc.tensor.matmul(wps[:, 0:32], lhsT=warm, rhs=warm_rhs32,
                                 start=True, stop=True)
            bw = 256 if t < 2 else 512
            for h in range(H):
                ps = psg.tile([128, 8, 128], F32, tag="pre", name=f"ps{t}_{h}")
                for blk in range(1024 // bw):
                    nc.tensor.matmul(
                        ps[:, bw // 128 * blk : bw // 128 * (blk + 1), :],
                        lhsT=wsl(0, h),
                        rhs=_ktjump(big_sb, PD0 + base + bw * blk, DAB, bw),
                        start=True, stop=False, perf_mode=DR)
                for blk in range(1024 // bw):
                    nc.tensor.matmul(
                        ps[:, bw // 128 * blk : bw // 128 * (blk + 1), :],
                        lhsT=wsl(1, h),
                        rhs=_ktjump(big_sb, PD1 + base + bw * blk, DAB, bw),
                        start=False, stop=True, perf_mode=DR)
                g_t = gpool.tile([128, 8, 128], BF16, tag="g", name=f"g{t}_{h}")
                nc.scalar.activation(out=g_t, in_=ps, func=GELU,
                                     bias=b1_ap(h), scale=1.0)
                pending.append((g_t, t, h))
                if len(pending) > 2:
                    emit_scores(*pending.pop(0))
        while pending:
            emit_scores(*pending.pop(0))

        # ---------- epilogue (per head, pipelined across engines) ----------
        av = psepi.tile([IBLK, H, HS + 1], F32, tag="av", name="av")
        for h in range(H):
            nc.scalar.activation(out=pt_sb[:, :, h, :], in_=score_ps[:, :, h, :],
                                 func=EXP, bias=float(b2_scaled[h]), scale=SCALE)
            mk = bass.AP(tensor=msl.tensor, offset=msl.offset,
                         ap=[list(msl.ap[0]), [64, 2], [1, 64]])
            nc.vector.tensor_mul(pt_sb[:, :, h, :], pt_sb[:, :, h, :], mk)
            for jc in range(2):
                nc.tensor.matmul(av[:, h, :], lhsT=pt_sb[:, jc, h, :],
                                 rhs=v_sb[:, jc, h, :],
                                 start=(jc == 0), stop=(jc == 1))
            nc.vector.reciprocal(out=recip[:, h, :], in_=av[:, h, HS : HS + 1])
            rb = recip[:, h, 0:1]
            recip_bc = bass.AP(tensor=rb.tensor, offset=rb.offset,
                               ap=[list(rb.ap[0]), [0, HS]])
            nc.vector.tensor_mul(final_sb[:, HS * h : HS * (h + 1)],
                                 av[:, h, 0:HS], recip_bc)
            nc.sync.dma_start(out=out[:, HS * h : HS * (h + 1)],
                              in_=final_sb[:, HS * h : HS * (h + 1)])

    return nc


def _prep_core(x1t_b, pd_b, k):
    """Build the per-core pd0/pd1/x1k/x1q streams (fp8) + mask, residue k."""
    f8 = ml_dtypes.float8_e4m3fn
    qsel = 4 * np.arange(IBLK) + k
    arr = pd_b[qsel].transpose(2, 0, 1)            # (256 c2, 64 q, 256 j)
    pd0 = np.concatenate(
        [arr[0:128, q, 0 : _ext(q)] for q in STREAM_Q], axis=1)
    pd1 = np.concatenate(
        [arr[128:256, q, 0 : _ext(q)] for q in STREAM_Q], axis=1)
    x1k = np.concatenate(
        [x1t_b[:, 0 : _ext(q)] for q in STREAM_Q], axis=1)
    x1q = np.repeat(x1t_b[:, qsel[STREAM_Q]],
                    [_ext(q) for q in STREAM_Q], axis=1)
    bigc = np.concatenate([pd0, pd1, x1k, x1q], axis=1).astype(f8)
    jidx = np.arange(128)[:, None, None] + np.array([0, 128])[None, :, None]
    mask = (jidx <= (4 * np.arange(IBLK) + k)[None, None, :]).astype(
        ml_dtypes.bfloat16)
    return bigc, mask


def kernel(**inputs):
    x = np.asarray(inputs["x"], np.float32)
    st = np.asarray(inputs["st_pos_emb"], np.float32)
    pd = np.asarray(inputs["pos_dist_emb"], np.float32)
    W1 = np.asarray(inputs["W1"], np.float32)
    b1 = np.asarray(inputs["b1"], np.float32)
    W2 = np.asarray(inputs["W2"], np.float32)
    b2 = np.asarray(inputs["b2"], np.float32)
    Wv = np.asarray(inputs["Wv"], np.float32)
    bv = np.asarray(inputs["bv"], np.float32)

    bf = ml_dtypes.bfloat16
    f8 = ml_dtypes.float8_e4m3fn
    x1 = x + st[None]                                    # (B, T, C)
    x1t_b = np.ascontiguousarray(x1.transpose(0, 2, 1))  # (B, C, T)

    W1k = W1[:, :C, :]                                   # (H, C, C)
    W1q = W1[:, C : 2 * C, :]
    W1p = W1[:, 2 * C :, :]                              # (H, 2C, C)
    # head-first stationary pack: per head [a-kt0 | a-kt1 | b-kt0 | b-kt1],
    # i.e. [W1p_lo | W1k | W1p_hi | W1q], each (c=128, m=128).
    wab_a = np.concatenate(
        [np.concatenate([W1p[h, 0:128], W1k[h], W1p[h, 128:256], W1q[h]],
                        axis=1)[:, None, :]
         for h in range(H)], axis=1).reshape(C, 2048).astype(f8)
    w2_a = np.ascontiguousarray(W2.T).astype(bf)         # (C, H)
    b1_a = np.ascontiguousarray(b1.T)                    # (C, H)
    wv_a = Wv.transpose(1, 0, 2).reshape(C, H * HS).astype(bf)

    key = tuple(float(v) * SCALE for v in b2)
    if key not in _build_cache:
        _build_cache[key] = _build(key)
    nc = _build_cache[key]

    in_maps = []
    for core in range(NCORES):
        b, k = divmod(core, 4)
        bigc, mask = _prep_core(x1t_b[b], pd[b], k)
        cpack = np.concatenate(
            [w2_a, wv_a, x[b].T.astype(bf), mask.reshape(128, 128)], axis=1)
        wabq = np.concatenate(
            [wab_a, w2_a.view(f8), b1_a.view(f8)], axis=1)
        in_maps.append({
            "big": bigc, "wabq": np.ascontiguousarray(wabq),
            "cpack": np.ascontiguousarray(cpack),
        })

    _install_wait_legalizer()
    res = run_bass_kernel_spmd(nc, in_maps, core_ids=list(range(NCORES)))
    outp = np.zeros((B, T, H * HS), np.float32)
    for core in range(NCORES):
        b, k = divmod(core, 4)
        outp[b, 4 * np.arange(IBLK) + k] = res.results[core]["out"]
    outp += bv.reshape(-1)[None, None, :]
    return outp


# revision 33
# speedup vs baseline: 1.0417x; 1.0417x over previous
"""Bass/Tile TRN2 kernel: pairwise-MLP multi-head attention (B=2,T=256,C=128,H=4,HS=32).

Sharding: 8 cores = (batch b in {0,1}) x (query residue k in {0..3}); core
(b, k) owns the 64 queries i == k (mod 4), so every core sees the same mix of
causal extents and the SPMD program is identical across cores.

Causal extents are rounded up to multiples of 32: local query q (global
i = 4q + k) computes E_q = 32*(q//8 + 1) key columns.  All (q, j) pairs are
packed into one flat stream (9216 columns per head-side), so every DoubleRow
pre-matmul block uses the same kt-jump delta:

  pd0cat | pd1cat | x1kcat | x1qcat    (big_sb, fp8; x1k/x1q built on-chip)

Per-core dataflow:
  pre[c,(q,j)] = fp8 DoubleRow matmuls: (W1p_lo | W1k) @ (pd0 | x1k) +
                 (W1p_hi | W1q) @ (pd1 | x1q), 256 rows per PE instruction.
  g = gelu(pre + b1[h])                        (ScalarE, psum -> sbuf bf16)
  score_t[j,q] = g_chunk.T @ w2[h]             (PE, g stationary, bf16)
  P_t = exp(scale*score_t + b2*scale) * mask   (ScalarE + DVE, per head)
  out[q,:] = P_t.T @ [v | 1]; out /= Z         (PE; Z rides as v's 33rd column)

Startup: gelu act-table + PE HAM prewarmed during the input DMA; weights are
packed head-first so head 0's stationaries arrive in the first small chunk.
"""

import sys
from contextlib import ExitStack

import numpy as np

for _p in ("/opt/trn_rl_repo", "/root/.axon_site/_ro/trn_rl_repo"):
    if _p not in sys.path:
        sys.path.append(_p)

import ml_dtypes

import concourse.bass as bass
import concourse.mybir as mybir
import concourse.tile as tile
from concourse.bass_utils import run_bass_kernel_spmd

B, T, C = 2, 256, 128
H, HS = 4, 32
IBLK = 64            # queries per core
NCORES = 8
SCALE = float(C) ** -0.5

F32 = mybir.dt.float32
BF16 = mybir.dt.bfloat16
F8 = mybir.dt.float8e4
DR = mybir.MatmulPerfMode.DoubleRow

GELU = mybir.ActivationFunctionType.Gelu
EXP = mybir.ActivationFunctionType.Exp

# ---- stream layout ----------------------------------------------------
# class c (0..7): local queries 8c..8c+7, extent E = 32*(c+1).
# tiles of 1024 stream columns; no query straddles a tile.
TILE_QS = [
    [56, 57, 58, 59],
    [60, 61, 62, 63],
    [24, 25, 26, 27, 28, 29, 30, 31],
    [48, 49, 50, 51, 0, 1, 2, 3],
    [52, 53, 54, 55, 4, 5, 6, 7],
    [40, 41, 42, 43, 8, 9, 10, 11],
    [44, 45, 46, 47, 12, 13, 14, 15],
    [32, 33, 34, 35, 16, 17, 18, 19],
    [36, 37, 38, 39, 20, 21, 22, 23],
]
NT = len(TILE_QS)


def _ext(q):
    return 32 * (q // 8 + 1)


STREAM_Q = [q for tq in TILE_QS for q in tq]
NS = sum(_ext(q) for q in STREAM_Q)          # 9216 stream cols per side
assert NS == 9216
# per-tile metadata: (q, E, colstart within tile)
TILE_META = []
for tq in TILE_QS:
    pos, meta = 0, []
    for q in tq:
        meta.append((q, _ext(q), pos))
        pos += _ext(q)
    assert pos == 1024
    TILE_META.append(meta)

PD0 = 0
PD1 = NS
XK = 2 * NS
XQ = 3 * NS
NBIG = 4 * NS
DAB = XK - PD0           # 18432: kt-jump delta for both matmuls

# wq layout: wab head-first (c, h, [a/b, kt], m) | w2 (8 fp8 = 4 bf16) |
# b1 (16 fp8 = 4 f32)
OW2 = 2048
OB1 = OW2 + 2 * H
NWQ = OB1 + 4 * H

_build_cache = {}


def _legalize_single_wait(bir_json):
    """Split multi-wait instructions into single-wait NoOps + instruction.

    This walrus build's codegen (setupSyncWait) accepts at most one sem wait
    per ISA struct, but Tile's sem-assignment attaches wait *lists*.  Waits
    are ANDed and executed in order by the issuing sequencer, so hoisting all
    but one onto same-engine NoOps immediately before is semantically
    identical.
    """
    import json as _json

    m = _json.loads(bir_json)
    for fn in m.get("functions", []):
        for blk in fn.get("blocks", []):
            new = []
            for ins in blk.get("instructions", []):
                si = ins.get("sync_info")
                waits = (si or {}).get("on_wait") or []
                if len(waits) > 1:
                    for k, w in enumerate(waits[:-1]):
                        nop = {
                            "debug": ins.get("debug", 0),
                            "engine": ins["engine"],
                            "ins": [],
                            "name": f"{ins['name']}-ws{k}",
                            "opcode": "NoOp",
                            "outs": [],
                            "sync_info": {"on_wait": [w], "on_update": []},
                        }
                        new.append(nop)
                    si = dict(si)
                    si["on_wait"] = [waits[-1]]
                    ins = dict(ins)
                    ins["sync_info"] = si
                new.append(ins)
            blk["instructions"] = new
    return _json.dumps(m).encode()


def _install_wait_legalizer():
    from concourse import bass2jax as _b2j
    from concourse import bass_utils as _bu

    if getattr(_b2j, "_single_wait_patched", False):
        return
    _orig = _bu.compile_bir_kernel

    def _patched(bir_json, tmpdir, neff_name="file.neff"):
        return _orig(_legalize_single_wait(bir_json), tmpdir, neff_name)

    _b2j.compile_bir_kernel = _patched
    _b2j._single_wait_patched = True


def _ktjump(tile_ap, off, delta, ncols):
    """rhs AP [128][kt: stride delta, 2][1, ncols] rooted at column `off`."""
    sl = tile_ap[:, off : off + 1]
    return bass.AP(
        tensor=sl.tensor,
        offset=sl.offset,
        ap=[list(sl.ap[0]), [delta, 2], [1, ncols]],
    )


def _flat(t, lo, n):
    """[128, n] view of columns [lo, lo+n) of a [128, 8, 128] sbuf tile."""
    sl = t[:, lo // 128, lo % 128 : lo % 128 + 1]
    return bass.AP(tensor=sl.tensor, offset=sl.offset,
                   ap=[list(sl.ap[0]), [1, n]])


def _build(b2_scaled):
    nc = bass.Bass()

    big = nc.dram_tensor("big", (128, NBIG), F8, kind="ExternalInput")
    wabq = nc.dram_tensor("wabq", (128, NWQ), F8, kind="ExternalInput")
    cpack = nc.dram_tensor("cpack", (128, 516), BF16, kind="ExternalInput")
    out = nc.dram_tensor("out", (IBLK, H * HS), F32, kind="ExternalOutput")

    with tile.TileContext(nc) as tc, ExitStack() as ctx:
        const = ctx.enter_context(tc.tile_pool(name="const", bufs=1))
        gpool = ctx.enter_context(tc.tile_pool(name="gpool", bufs=4))
        psg = ctx.enter_context(tc.tile_pool(name="psg", bufs=2, space="PSUM"))
        pssc = ctx.enter_context(tc.tile_pool(name="pssc", bufs=1, space="PSUM"))
        psepi = ctx.enter_context(tc.tile_pool(name="psepi", bufs=1, space="PSUM"))
        pswarm = ctx.enter_context(tc.tile_pool(name="pswarm", bufs=1, space="PSUM"))

        wq_sb = const.tile([128, NWQ], F8)
        cp_sb = const.tile([128, 516], BF16)
        big_sb = const.tile([128, NBIG], F8)

        # ---------- prewarm seeds (idle engines at t=0) ----------
        warm = const.tile([128, 16], BF16)
        wscr = const.tile([128, 16], BF16)
        nc.vector.memset(warm, 1.0)
        # gelu ACT_TABLE_LOAD fires before this, overlapping the input DMA
        nc.scalar.activation(out=wscr, in_=warm, func=GELU, scale=1.0)

        # ---------- input DMA: 3 queues, chunks ordered by first use ------
        # scalar: w2/b1 mini-chunk, head-0 weights, rest, cpack
        nc.scalar.dma_start(out=wq_sb[:, OW2:NWQ], in_=wabq[:, OW2:NWQ])
        nc.scalar.dma_start(out=wq_sb[:, 0:512], in_=wabq[:, 0:512])
        nc.scalar.dma_start(out=wq_sb[:, 512:2048], in_=wabq[:, 512:2048])
        nc.scalar.dma_start(out=cp_sb, in_=cpack[:])

        # sync (HW queue): pd0 + x1k; gpsimd (SW queue): pd1 + x1q, tile-major
        chunks = ((0, 512), (512, 1024), (1024, 2048), (2048, 3072),
                  (3072, 4608), (4608, 6656), (6656, 9216))
        for lo, hi in chunks:
            nc.sync.dma_start(out=big_sb[:, PD0 + lo : PD0 + hi],
                              in_=big[:, PD0 + lo : PD0 + hi])
            nc.gpsimd.dma_start(out=big_sb[:, PD1 + lo : PD1 + hi],
                                in_=big[:, PD1 + lo : PD1 + hi])
            nc.sync.dma_start(out=big_sb[:, XK + lo : XK + hi],
                              in_=big[:, XK + lo : XK + hi])
            nc.gpsimd.dma_start(out=big_sb[:, XQ + lo : XQ + hi],
                                in_=big[:, XQ + lo : XQ + hi])

        # ---------- PE HAM warm-up ----------
        wps = pswarm.tile([16, 512], F32)
        wsl16 = warm[:, 0:1]
        warm_rhs = bass.AP(tensor=wsl16.tensor, offset=wsl16.offset,
                           ap=[list(wsl16.ap[0]), [0, 512]])
        for _ in range(10):
            nc.tensor.matmul(wps, lhsT=warm, rhs=warm_rhs, start=True, stop=True)

        msl = cp_sb[:, 388:389]

        def w2_ap(h):
            o = OW2 + 2 * h
            return wq_sb[:, o : o + 2].bitcast(BF16)

        def b1_ap(h):
            o = OB1 + 4 * h
            return wq_sb[:, o : o + 4].bitcast(F32)

        def wsl(ab, h):
            """lhsT view [c, kt, 128] of head h's a/b weight pack."""
            o = h * 512 + ab * 256
            sl = wq_sb[:, o : o + 1]
            return bass.AP(tensor=sl.tensor, offset=sl.offset,
                           ap=[list(sl.ap[0]), [128, 2], [1, 128]])

        pt_sb = const.tile([128, 2, H, IBLK], BF16)
        v_sb = const.tile([128, 2, H, HS + 1], BF16)
        recip = const.tile([IBLK, H, 1], F32)
        final_sb = const.tile([IBLK, H * HS], F32)

        # score accumulator [j%128, jb, h, i] - 1 PSUM bank, memset so the
        # never-written j-tiles of short queries exp() to a finite value.
        score_ps = pssc.tile([128, 2, H, IBLK], F32)
        nc.vector.memset(score_ps, 0.0)
        nc.vector.memset(v_sb[:, :, :, HS : HS + 1], 1.0)

        # ---------- v = x @ Wv (+ ones column for Z) ----------
        for jc in range(2):
            v_ps = psepi.tile([128, H, HS], F32, tag="vps", name=f"v{jc}")
            nc.tensor.matmul(v_ps, lhsT=cp_sb[:, 132 + jc * 128 : 260 + jc * 128],
                             rhs=cp_sb[:, 4:132], start=True, stop=True)
            nc.vector.tensor_copy(v_sb[:, jc, :, 0:HS], v_ps)

        # ---------- main loop ----------
        pending = []

        def emit_scores(g_t, t, h):
            # chunks padded to M=128 where possible (garbage partitions land
            # beyond the causal extent and are killed by the mask) so every
            # LDWEIGHTS is a full 128-col FWL load.
            for q, E, cs in TILE_META[t]:
                m0 = min(128, 1024 - cs)
                nc.tensor.matmul(score_ps[0:m0, 0, h, q : q + 1],
                                 lhsT=_flat(g_t, cs, m0), rhs=w2_ap(h),
                                 start=True, stop=True)
                if E > 128:
                    m1 = min(128, 1024 - cs - 128)
                    nc.tensor.matmul(score_ps[0:m1, 1, h, q : q + 1],
                                     lhsT=_flat(g_t, cs + 128, m1),
                                     rhs=w2_ap(h), start=True, stop=True)

        warm_rhs32 = bass.AP(tensor=wsl16.tensor, offset=wsl16.offset,
                             ap=[list(wsl16.ap[0]), [0, 32]])
        for t in range(NT):
            base = 1024 * t
            # keep-alive: tiny dep-free matmul so a data stall here cannot
            # leave the PE idle for a full HAM MID window (clock re-throttle)
            nc.tensor.matmul(wps[:, 0:32], lhsT=warm, rhs=warm_rhs32,
                             start=True, stop=True)
            for h in range(H):
                ps = psg.tile([128, 8, 128], F32, tag="pre", name=f"ps{t}_{h}")
                for blk in range(2):
                    nc.tensor.matmul(
                        ps[:, 4 * blk : 4 * blk + 4, :], lhsT=wsl(0, h),
                        rhs=_ktjump(big_sb, PD0 + base + 512 * blk, DAB, 512),
                        start=True, stop=False, perf_mode=DR)
                for blk in range(2):
                    nc.tensor.matmul(
                        ps[:, 4 * blk : 4 * blk + 4, :], lhsT=wsl(1, h),
                        rhs=_ktjump(big_sb, PD1 + base + 512 * blk, DAB, 512),
                        start=False, stop=True, perf_mode=DR)
                g_t = gpool.tile([128, 8, 128], BF16, tag="g", name=f"g{t}_{h}")
                nc.scalar.activation(out=g_t, in_=ps, func=GELU,
                                     bias=b1_ap(h), scale=1.0)
                pending.append((g_t, t, h))
                if len(pending) > 2:
                    emit_scores(*pending.pop(0))
        while pending:
            emit_scores(*pending.pop(0))

        # ---------- epilogue (per head, pipelined across engines) ----------
        av = psepi.tile([IBLK, H, HS + 1], F32, tag="av", name="av")
        for h in range(H):
            nc.scalar.activation(out=pt_sb[:, :, h, :], in_=score_ps[:, :, h, :],
                                 func=EXP, bias=float(b2_scaled[h]), scale=SCALE)
            mk = bass.AP(tensor=msl.tensor, offset=msl.offset,
                         ap=[list(msl.ap[0]), [64, 2], [1, 64]])
            nc.vector.tensor_mul(pt_sb[:, :, h, :], pt_sb[:, :, h, :], mk)
            for jc in range(2):
                nc.tensor.matmul(av[:, h, :], lhsT=pt_sb[:, jc, h, :],
                                 rhs=v_sb[:, jc, h, :],
                                 start=(jc == 0), stop=(jc == 1))
        nc.vector.reciprocal(out=recip, in_=av[:, :, HS : HS + 1])
        rb = recip[:, :, 0:1]
        recip_bc = bass.AP(tensor=rb.tensor, offset=rb.offset,
                           ap=[list(rb.ap[0]), [1, H], [0, HS]])
        fview = bass.AP(tensor=final_sb.tensor, offset=final_sb.offset,
                        ap=[list(final_sb.ap[0]), [HS, H], [1, HS]])
        nc.vector.tensor_mul(fview, av[:, :, 0:HS], recip_bc)
        nc.sync.dma_start(out=out[:], in_=final_sb)

    return nc


def _prep_core(x1t_b, pd_b, k):
    """Build the per-core pd0/pd1/x1k/x1q streams (fp8) + mask, residue k."""
    f8 = ml_dtypes.float8_e4m3fn
    qsel = 4 * np.arange(IBLK) + k
    arr = pd_b[qsel].transpose(2, 0, 1)            # (256 c2, 64 q, 256 j)
    pd0 = np.concatenate(
        [arr[0:128, q, 0 : _ext(q)] for q in STREAM_Q], axis=1)
    pd1 = np.concatenate(
        [arr[128:256, q, 0 : _ext(q)] for q in STREAM_Q], axis=1)
    x1k = np.concatenate(
        [x1t_b[:, 0 : _ext(q)] for q in STREAM_Q], axis=1)
    x1q = np.repeat(x1t_b[:, qsel[STREAM_Q]],
                    [_ext(q) for q in STREAM_Q], axis=1)
    bigc = np.concatenate([pd0, pd1, x1k, x1q], axis=1).astype(f8)
    jidx = np.arange(128)[:, None, None] + np.array([0, 128])[None, :, None]
    mask = (jidx <= (4 * np.arange(IBLK) + k)[None, None, :]).astype(
        ml_dtypes.bfloat16)
    return bigc, mask


def kernel(**inputs):
    x = np.asarray(inputs["x"], np.float32)
    st = np.asarray(inputs["st_pos_emb"], np.float32)
    pd = np.asarray(inputs["pos_dist_emb"], np.float32)
    W1 = np.asarray(inputs["W1"], np.float32)
    b1 = np.asarray(inputs["b1"], np.float32)
    W2 = np.asarray(inputs["W2"], np.float32)
    b2 = np.asarray(inputs["b2"], np.float32)
    Wv = np.asarray(inputs["Wv"], np.float32)
    bv = np.asarray(inputs["bv"], np.float32)

    bf = ml_dtypes.bfloat16
    f8 = ml_dtypes.float8_e4m3fn
    x1 = x + st[None]                                    # (B, T, C)
    x1t_b = np.ascontiguousarray(x1.transpose(0, 2, 1))  # (B, C, T)

    W1k = W1[:, :C, :]                                   # (H, C, C)
    W1q = W1[:, C : 2 * C, :]
    W1p = W1[:, 2 * C :, :]                              # (H, 2C, C)
    # head-first stationary pack: per head [a-kt0 | a-kt1 | b-kt0 | b-kt1],
    # i.e. [W1p_lo | W1k | W1p_hi | W1q], each (c=128, m=128).
    wab_a = np.concatenate(
        [np.concatenate([W1p[h, 0:128], W1k[h], W1p[h, 128:256], W1q[h]],
                        axis=1)[:, None, :]
         for h in range(H)], axis=1).reshape(C, 2048).astype(f8)
    w2_a = np.ascontiguousarray(W2.T).astype(bf)         # (C, H)
    b1_a = np.ascontiguousarray(b1.T)                    # (C, H)
    wv_a = Wv.transpose(1, 0, 2).reshape(C, H * HS).astype(bf)

    key = tuple(float(v) * SCALE for v in b2)
    if key not in _build_cache:
        _build_cache[key] = _build(key)
    nc = _build_cache[key]

    in_maps = []
    for core in range(NCORES):
        b, k = divmod(core, 4)
        bigc, mask = _prep_core(x1t_b[b], pd[b], k)
        cpack = np.concatenate(
            [w2_a, wv_a, x[b].T.astype(bf), mask.reshape(128, 128)], axis=1)
        wabq = np.concatenate(
            [wab_a, w2_a.view(f8), b1_a.view(f8)], axis=1)
        in_maps.append({
            "big": bigc, "wabq": np.ascontiguousarray(wabq),
            "cpack": np.ascontiguousarray(cpack),
        })

    _install_wait_legalizer()
    res = run_bass_kernel_spmd(nc, in_maps, core_ids=list(range(NCORES)))
    outp = np.zeros((B, T, H * HS), np.float32)
    for core in range(NCORES):
        b, k = divmod(core, 4)
        outp[b, 4 * np.arange(IBLK) + k] = res.results[core]["out"]
    outp += bv.reshape(-1)[None, None, :]
    return outp
